# revision 9
# baseline (speedup 1.0000x reference)
"""Trainium2 Bass kernel for nn_NodeModel (GNN message passing).

reference:
    agg = segment_sum(edge_attr, edge_index[0], num_segments=100000)   # [N, 64]
    h = concat([x, agg, u[v_indices]], axis=1)                         # [N, 256]
    out = relu(h @ W1 + b1) @ W2 + b2                                  # [N, 128]

Strategy (8 NeuronCores, SPMD, no collectives):
  - Shard nodes across cores (12500/core); shard edges by destination-node
    partition (host buckets+sorts edges by the core/block owning their row).
  - Within a core, nodes are processed in blocks of 128. Edges are sorted by
    row, grouped per block, padded to T tiles of 128 edges.
  - segment_sum on device: per 128-edge tile, build a one-hot matrix
    P[e, m] = (row_local[e] == m) with DVE/GPSIMD tensor_scalar(is_equal),
    then TensorE matmul:  aggT_hilo += ea_hilo.T @ P  accumulated in PSUM.
  - edge_attr is split hi/lo bf16 (exact to ~1e-5, same total bytes as fp32).
    The hi and lo partial sums land on PSUM partitions 0-63 / 64-127; they
    are merged for free inside the MLP by duplicating W1's agg-rows.
  - MLP runs feature-major (transposed) with fp32r matmuls, N=512 node
    groups. x and u[v_indices] are pre-transposed on host; the output is
    produced transposed and un-transposed on host.
"""

import sys

sys.path.insert(0, "/opt/trn_rl_repo")

import numpy as np
import ml_dtypes

import concourse.bass as bass
import concourse.mybir as mybir
from concourse import bacc, tile
from concourse.bass_utils import run_bass_kernel_spmd

bf16 = ml_dtypes.bfloat16

D_X, D_E, D_U = 128, 64, 64
D_HID, D_OUT = 256, 128
NB = 128  # nodes per block

FULL_CFG = dict(n_cores=8, n_nodes=100000, npc=12500, blocks=98, group=4)

_cache = {}


def _build_nc(T, blocks, npad, group, n_cores=8):
    """Build the SPMD Bass program. T = edge tiles per block."""
    nc = bacc.Bacc(
        "TRN2", target_bir_lowering=False, debug=False, num_devices=n_cores
    )
    f32, rf32, b16 = mybir.dt.float32, mybir.dt.float32r, mybir.dt.bfloat16

    ea_in = nc.declare_dram_parameter("ea", [blocks, T, 128, 128], b16, isOutput=False)
    # idx carries a bf16 iota [128,128] packed into its first 64 f32 columns
    idx_in = nc.declare_dram_parameter("idx", [128, 64 + blocks * T], f32, isOutput=False)
    xT_in = nc.declare_dram_parameter("xT", [128, npad], rf32, isOutput=False)
    ugT_in = nc.declare_dram_parameter("ugT", [64, npad], rf32, isOutput=False)
    # weight layouts are partition-major: [K-part, mh, M]
    w1x_in = nc.declare_dram_parameter("w1x", [128, 2, 128], rf32, isOutput=False)
    w1a_in = nc.declare_dram_parameter("w1a", [128, 2, 128], rf32, isOutput=False)
    w1u_in = nc.declare_dram_parameter("w1u", [64, 2, 128], rf32, isOutput=False)
    w2_in = nc.declare_dram_parameter("w2", [128, 2, 128], rf32, isOutput=False)
    b1_in = nc.declare_dram_parameter("b1", [128, 2], f32, isOutput=False)
    b2_in = nc.declare_dram_parameter("b2", [128, 1], f32, isOutput=False)
    outT = nc.declare_dram_parameter("outT", [128, npad], f32, isOutput=True)

    n_groups = (blocks + group - 1) // group

    with tile.TileContext(nc) as tc:
        with (
            tc.tile_pool(name="const", bufs=1) as cpool,
            tc.tile_pool(name="xt", bufs=1) as xpool,
            tc.tile_pool(name="ea", bufs=3) as eapool,
            tc.tile_pool(name="p", bufs=6) as ppool,
            tc.tile_pool(name="hag", bufs=2) as hagpool,
            tc.tile_pool(name="ug", bufs=2) as ugpool,
            tc.tile_pool(name="h1", bufs=4) as h1pool,
            tc.tile_pool(name="outs", bufs=2) as opool,
            tc.tile_pool(name="ps_agg", bufs=2, space="PSUM") as agg_ps_pool,
            tc.tile_pool(name="ps_o1", bufs=4, space="PSUM") as o1_ps_pool,
            tc.tile_pool(name="ps_o2", bufs=2, space="PSUM") as o2_ps_pool,
        ):
            # ---- constants / resident tensors ----
            idx_t = cpool.tile([128, 64 + blocks * T], f32, tag="idx")
            nc.sync.dma_start(idx_t[:], idx_in[:])
            iota_ap = idx_t[:, 0:64].bitcast(b16)  # [128, 128] bf16 iota
            w1x_t = cpool.tile([128, 2, 128], rf32, tag="w1x")
            nc.sync.dma_start(w1x_t[:], w1x_in[:])
            w1a_t = cpool.tile([128, 2, 128], rf32, tag="w1a")
            nc.sync.dma_start(w1a_t[:], w1a_in[:])
            w1u_t = cpool.tile([64, 2, 128], rf32, tag="w1u")
            nc.sync.dma_start(w1u_t[:], w1u_in[:])
            w2_t = cpool.tile([128, 2, 128], rf32, tag="w2")
            nc.sync.dma_start(w2_t[:], w2_in[:])
            b1_t = cpool.tile([128, 2], f32, tag="b1")
            nc.sync.dma_start(b1_t[:], b1_in[:])
            b2_t = cpool.tile([128, 1], f32, tag="b2")
            nc.sync.dma_start(b2_t[:], b2_in[:])

            xT_t = xpool.tile([128, npad], rf32, tag="xT")
            # load x in chunks so early groups can start sooner
            xchunk = 8 * NB
            for s in range(0, npad, xchunk):
                e = min(s + xchunk, npad)
                nc.sync.dma_start(xT_t[:, s:e], xT_in[:, s:e])

            hag_tiles = {}
            # ---- edge scatter-add per block ----
            for b in range(blocks):
                g, bi = divmod(b, group)
                if bi == 0:
                    gw = min(group, blocks - g * group) * NB
                    hag_tiles[g] = hagpool.tile(
                        [128, group * NB], rf32, tag="hag", name=f"hag{g}"
                    )
                ea_t = eapool.tile([128, T, 128], b16, tag="ea")
                nc.sync.dma_start(ea_t[:], ea_in[b].rearrange("t p m -> p t m"))
                agg_ps = agg_ps_pool.tile([128, NB], f32, tag="agg")
                for t in range(T):
                    p_t = ppool.tile([128, NB], b16, tag="p")
                    eng = nc.vector
                    eng.tensor_scalar(
                        out=p_t[:],
                        in0=iota_ap,
                        scalar1=idx_t[:, 64 + b * T + t : 64 + b * T + t + 1],
                        scalar2=None,
                        op0=mybir.AluOpType.is_equal,
                    )
                    nc.tensor.matmul(
                        agg_ps[:],
                        ea_t[:, t, :],
                        p_t[:],
                        start=(t == 0),
                        stop=(t == T - 1),
                    )
                # move [aggT_hi ; aggT_lo] into the MLP's K-chunk staging tile
                nc.scalar.activation(
                    out=hag_tiles[g][:, bi * NB : (bi + 1) * NB],
                    in_=agg_ps[:],
                    func=mybir.ActivationFunctionType.Copy,
                )

            # ---- MLP per group of blocks (feature-major) ----
            for g in range(n_groups):
                s = g * group * NB
                gw = min(group * NB, npad - s)
                ug_t = ugpool.tile([64, group * NB], rf32, tag="ug")
                nc.sync.dma_start(ug_t[:, :gw], ugT_in[:, s : s + gw])
                hag = hag_tiles[g]
                h1_list = []
                for mh in range(2):
                    o1 = o1_ps_pool.tile([128, group * NB], f32, tag="o1")
                    nc.tensor.matmul(
                        o1[:, :gw], w1x_t[:, mh, :],
                        xT_t[:, s : s + gw],
                        start=True, stop=False,
                    )
                    nc.tensor.matmul(
                        o1[:, :gw], w1a_t[:, mh, :],
                        hag[:, :gw],
                        start=False, stop=False,
                    )
                    nc.tensor.matmul(
                        o1[:, :gw], w1u_t[:, mh, :],
                        ug_t[:, :gw],
                        start=False, stop=True,
                    )
                    h1 = h1pool.tile([128, group * NB], rf32, tag="h1")
                    nc.scalar.activation(
                        out=h1[:, :gw], in_=o1[:, :gw],
                        func=mybir.ActivationFunctionType.Relu,
                        bias=b1_t[:, mh : mh + 1],
                    )
                    h1_list.append(h1)
                o2 = o2_ps_pool.tile([128, group * NB], f32, tag="o2")
                for kh in range(2):
                    nc.tensor.matmul(
                        o2[:, :gw], w2_t[:, kh, :],
                        h1_list[kh][:, :gw],
                        start=(kh == 0), stop=(kh == 1),
                    )
                out_t = opool.tile([128, group * NB], f32, tag="outs")
                nc.scalar.activation(
                    out=out_t[:, :gw], in_=o2[:, :gw],
                    func=mybir.ActivationFunctionType.Identity,
                    bias=b2_t[:],
                )
                nc.scalar.dma_start(outT[:, s : s + gw], out_t[:, :gw])

    nc.compile()
    return nc


def _pack_inputs(x, edge_index, edge_attr, u, v_indices, W1, b1, W2, b2, cfg):
    """Host-side sharding: bucket + sort edges by destination node partition."""
    n_cores, npc, blocks = cfg["n_cores"], cfg["npc"], cfg["blocks"]
    n_nodes = cfg["n_nodes"]
    npad = blocks * NB
    row = np.asarray(edge_index[0], dtype=np.int64)
    ea = np.ascontiguousarray(np.asarray(edge_attr, dtype=np.float32))
    x = np.asarray(x, dtype=np.float32)
    u = np.asarray(u, dtype=np.float32)
    v_indices = np.asarray(v_indices, dtype=np.int64)
    W1 = np.asarray(W1, dtype=np.float32)
    W2 = np.asarray(W2, dtype=np.float32)
    b1 = np.asarray(b1, dtype=np.float32)
    b2 = np.asarray(b2, dtype=np.float32)
    d_e = ea.shape[1]

    order = np.argsort(row, kind="stable")
    row_s = row[order]
    ea_s = ea[order]
    hi = ea_s.astype(bf16)
    lo = (ea_s - hi.astype(np.float32)).astype(bf16)
    ea_hilo = np.concatenate([hi, lo], axis=1)  # [E, 2*d_e] bf16

    # block boundaries: core c block b covers nodes [npc*c + NB*b, +NB),
    # clipped to the core's node range.
    bases = (npc * np.arange(n_cores)[:, None] + NB * np.arange(blocks)[None, :]).ravel()
    core_hi = (npc * (1 + np.arange(n_cores))[:, None]).repeat(blocks, 1).ravel()
    starts = np.searchsorted(row_s, bases, side="left")
    ends = np.searchsorted(row_s, np.minimum(bases + NB, core_hi), side="left")
    cnts = ends - starts
    T = max(1, int(np.max((cnts + 127) // 128)))

    ea_pack = np.zeros((n_cores, blocks, T * 128, 2 * d_e), dtype=bf16)
    idx_pack = np.zeros((n_cores, blocks, T * 128), dtype=np.float32)
    k = 0
    for c in range(n_cores):
        for b in range(blocks):
            s, e = starts[k], ends[k]
            if e > s:
                ea_pack[c, b, : e - s] = ea_hilo[s:e]
                idx_pack[c, b, : e - s] = (row_s[s:e] - bases[k]).astype(np.float32)
            k += 1
    ea_pack = ea_pack.reshape(n_cores, blocks, T, 128, 2 * d_e)
    # idx layout: [core, 128 partitions, blocks*T]
    idx_pack = np.ascontiguousarray(
        idx_pack.reshape(n_cores, blocks, T, 128).transpose(0, 3, 1, 2)
    ).reshape(n_cores, 128, blocks * T)

    iota = np.broadcast_to(np.arange(128, dtype=np.float32), (128, 128)).astype(bf16)
    iota_f32 = np.ascontiguousarray(iota).view(np.float32)  # [128, 64]
    uT = u.T  # [d_u, n_graphs]

    # weights, partition-major [K, mh, M]
    w1x = np.ascontiguousarray(W1[:D_X].reshape(D_X, 2, 128))
    w1a_single = W1[D_X : D_X + d_e]                       # [64, 256]
    w1a_dup = np.concatenate([w1a_single, w1a_single], 0)  # [128, 256] hi|lo dup
    w1a = np.ascontiguousarray(w1a_dup.reshape(128, 2, 128))
    w1u = np.ascontiguousarray(W1[D_X + d_e :].reshape(D_U, 2, 128))
    w2 = np.ascontiguousarray(W2.reshape(2, 128, D_OUT).transpose(1, 0, 2))
    b1p = np.ascontiguousarray(b1.reshape(2, 128).T)
    b2p = np.ascontiguousarray(b2.reshape(128, 1))

    in_maps = []
    for c in range(n_cores):
        lo_n, hi_n = npc * c, min(npc * (c + 1), n_nodes)
        xT = np.zeros((D_X, npad), dtype=np.float32)
        xT[:, : hi_n - lo_n] = x[lo_n:hi_n].T
        ugT = np.zeros((D_U, npad), dtype=np.float32)
        ugT[:, : hi_n - lo_n] = uT[:, v_indices[lo_n:hi_n]]
        in_maps.append({
            "ea": ea_pack[c],
            "idx": np.concatenate([iota_f32, idx_pack[c]], axis=1),
            "xT": xT,
            "ugT": ugT,
            "w1x": w1x,
            "w1a": w1a,
            "w1u": w1u,
            "w2": w2,
            "b1": b1p,
            "b2": b2p,
        })
    return in_maps, T


def _run(inputs, cfg, trace=False):
    in_maps, T = _pack_inputs(
        inputs["x"], inputs["edge_index"], inputs["edge_attr"], inputs["u"],
        inputs["v_indices"], inputs["W1"], inputs["b1"], inputs["W2"],
        inputs["b2"], cfg,
    )
    key = (T, cfg["blocks"], cfg["group"])
    if key not in _cache:
        _cache[key] = _build_nc(T, cfg["blocks"], cfg["blocks"] * NB, cfg["group"])
    nc = _cache[key]
    res = run_bass_kernel_spmd(nc, in_maps, list(range(cfg["n_cores"])), trace=trace)
    n_nodes, npc = cfg["n_nodes"], cfg["npc"]
    out = np.empty((n_nodes, D_OUT), dtype=np.float32)
    for c in range(cfg["n_cores"]):
        lo_n, hi_n = npc * c, min(npc * (c + 1), n_nodes)
        out[lo_n:hi_n] = res.results[c]["outT"].T[: hi_n - lo_n]
    _run.last_results = res
    return out


def kernel(x, edge_index, edge_attr, u, v_indices, W1, b1, W2, b2):
    inputs = dict(x=x, edge_index=edge_index, edge_attr=edge_attr, u=u,
                  v_indices=v_indices, W1=W1, b1=b1, W2=W2, b2=b2)
    return _run(inputs, FULL_CFG)
